# revision 36
# baseline (speedup 1.0000x reference)
"""Bahdanau-style attention kernel for Trainium2 (8 NeuronCores, data-parallel).

Computes, for each batch b:
    h_proj = hidden @ w_h^T + attn_b                  # [H]
    e_proj = enc[b] @ w_e^T                           # [L, H]
    energy = tanh(h_proj + e_proj)                    # [L, H]
    scores = energy @ v_w                             # [L]
    weights = softmax(scores)                         # [L]
    context[b] = weights @ enc[b]                     # [H]

Sharding: data-parallel over batch B=32 across 8 cores (4 batches/core).
Params are replicated. The softmax max-subtraction is skipped (scores are
bounded by sum|v| <= 32, exp is safe in fp32); the 1/Z normalization is
folded into the final context scaling.

The dominant e_proj GEMM runs in fp8e4m3 with perf_mode=DoubleRow (2
weights/PE cell, 256-deep contraction per pass -> ~2x matmul throughput).
w_e is host-scaled by 32 to clear the e4m3 subnormal range; the 1/32 is
folded into the tanh activation's scale.

The score partition-reduction uses the v-weighted energy accumulator as
the matmul *stationary* against a ones vector, which lands scores already
transposed ([l%128 on partitions]) so exp() writes the context-matmul
weights directly - no DRAM transpose bounce. The context matmul runs in
bf16 and packs the two h-halves into PE column groups 0/32
(tile_position col-tiling) so the two [1,512] matmuls run concurrently.
Z stays fp32 via the activation accumulator ([128,1] partials reduced at
batch finalization), so the softmax normalization is exact.

Built on bacc.Bacc so compile() runs the TRN2 wait-splitting passes
(move_matmul_waits_to_ldweights / generate_event_semaphores).
"""

import numpy as np

H = 1024
B = 32
L = 2048
NCORES = 8
BPC = B // NCORES          # batches per core = 4
KC = H // 128              # contraction chunks of 128 = 8
KC2 = KC // 2              # DoubleRow chunk pairs = 4
OC = H // 128              # output-feature chunks = 8
NLT = L // 512             # l-tiles of 512 = 4
NLCH = L // 128            # l-chunks of 128 = 16
W_SCALE = 32.0             # host-side w_e scaling (cleared by tanh scale)
CTX_W0 = 0.014             # ms: scheduler-model floor for first ctx block
CTX_MS = 0.0070            # ms: per-slab increment of the ctx floor

_CACHED_NC = None


def _build_kernel():
    from contextlib import ExitStack

    import concourse.tile as tile
    from concourse import bacc
    from concourse import mybir
    from concourse.masks import make_identity

    f32 = mybir.dt.float32
    f32r = mybir.dt.float32r
    f8 = mybir.dt.float8e4
    bf16 = mybir.dt.bfloat16
    AF = mybir.ActivationFunctionType
    DR = mybir.MatmulPerfMode.DoubleRow

    nc = bacc.Bacc("TRN2", target_bir_lowering=False, debug=False,
                   num_devices=NCORES)

    # all inputs host-laid-out so every DMA is contiguous per partition
    encT = nc.dram_tensor("encTr", [BPC, 128, NLT, KC, 512], f8,
                          kind="ExternalInput").ap()
    encN = nc.dram_tensor("encNr", [BPC, 128, NLT, 4, H], bf16,
                          kind="ExternalInput").ap()
    w_eT = nc.dram_tensor("wer", [128, KC, H], f8, kind="ExternalInput").ap()
    smallr = nc.dram_tensor("smallr", [128, OC + OC * BPC], f32,
                            kind="ExternalInput").ap()
    ctx_out = nc.dram_tensor("ctx", [BPC, H], f32, kind="ExternalOutput").ap()

    with tile.TileContext(nc) as tc, ExitStack() as ctx:
        consts = ctx.enter_context(tc.tile_pool(name="consts", bufs=1))
        encT_pool = ctx.enter_context(tc.tile_pool(name="encT", bufs=5))
        encN_pool = ctx.enter_context(tc.tile_pool(name="encN", bufs=3))
        en_pool = ctx.enter_context(tc.tile_pool(name="energy", bufs=6))
        small = ctx.enter_context(tc.tile_pool(name="small", bufs=2))
        expwT_pool = ctx.enter_context(tc.tile_pool(name="expwT", bufs=2))

        # ---- constants ----
        # head loads are spread over all four free DMA rings (sync, vector,
        # scalar, gpsimd) so the first slab + weights land ~4x sooner; each
        # keeps multi-KB per-partition lines for packet efficiency.
        small_sb = consts.tile([128, OC + OC * BPC], f32)
        nc.gpsimd.dma_start(out=small_sb, in_=smallr)
        v_sb = small_sb[:, 0:OC]
        # h_proj + attn_b, host-folded: [128, OC, BPC]
        hproj_sb = small_sb[:, OC:].rearrange("p (o b) -> p o b", b=BPC)
        encTs_pre = encT_pool.tile([128, KC, 512], f8, tag="encTs",
                                   name="encTs_pre")
        nc.sync.dma_start(out=encTs_pre[:, 0:KC // 2, :],
                          in_=encT[0, :, 0, 0:KC // 2, :])
        nc.gpsimd.dma_start(out=encTs_pre[:, KC // 2:, :],
                            in_=encT[0, :, 0, KC // 2:, :])
        we_sb = consts.tile([128, KC, H], f8)            # w_e^T  [h-part, k, o]
        nc.scalar.dma_start(out=we_sb[:, 0:2, :], in_=w_eT[:, 0:2, :])
        nc.scalar.dma_start(out=we_sb[:, 2:KC // 2, :],
                            in_=w_eT[:, 2:KC // 2, :])
        nc.sync.dma_start(out=we_sb[:, KC // 2:KC // 2 + 2, :],
                          in_=w_eT[:, KC // 2:KC // 2 + 2, :])
        nc.gpsimd.dma_start(out=we_sb[:, KC // 2 + 2:, :],
                            in_=w_eT[:, KC // 2 + 2:, :])
        ident = consts.tile([128, 128], f32)
        make_identity(nc, ident)
        ones4_bf = consts.tile([128, 4], bf16)
        nc.vector.memset(ones4_bf, 1.0)
        # warmup weights via a fast vector-queue memset so the HAM warmup
        # isn't gated on make_identity riding the busy gpsimd queue
        warm_w = consts.tile([128, 128], f32)
        nc.vector.memset(warm_w, 0.5)

        with tc.tile_pool(name="pp_pro", bufs=1, space="PSUM") as pp_pro:
            # warm the PE HAM while the weight DMAs stream
            pwarm = pp_pro.tile([128, 128], f32, tag="pwarm")
            for w in range(42):
                nc.tensor.matmul(pwarm, warm_w, warm_w, start=True, stop=True,
                                 skip_group_check=True)

        pp_e = ctx.enter_context(tc.tile_pool(name="pp_e", bufs=6, space="PSUM"))
        pp_s = ctx.enter_context(tc.tile_pool(name="pp_s", bufs=1, space="PSUM"))
        pp_c = ctx.enter_context(tc.tile_pool(name="pp_c", bufs=1, space="PSUM"))

        # ---- main pipeline: flat stream of l-slabs across batches ----
        # ctx matmuls run one slab behind their scores; batch finalization
        # (Z reduce + scale + store) rides behind the next batch's first slab.
        state = {}

        def ctx_mms(b, lt, encNs):
            st = state[b]
            if st["pcs"] is None:
                st["pcs"] = pp_c.tile([128, 512], f32, tag="pc",
                                      name=f"pc{b}")
            for j in range(4):
                lc = lt * 4 + j
                for half in range(2):
                    nc.tensor.matmul(
                        st["pcs"][32 * half:32 * half + 1, :],
                        st["expwT"][:, lc:lc + 1],
                        encNs[:, j, half * 512:(half + 1) * 512],
                        start=(lc == 0), stop=(lc == NLCH - 1),
                        tile_position=(0, 32 * half),
                    )

        def finalize(b):
            st = state.pop(b)
            # Z = sum over [128, NLT] partials: free-reduce on DVE, then a
            # ones-matmul folds the 128 partitions into a [1,1] psum scalar.
            zp = small.tile([128, 1], f32, tag="zp", name=f"zp{b}")
            nc.vector.reduce_sum(zp, st["zacc"], axis=mybir.AxisListType.X)
            # flip the 128 Z-partials onto partition 0 with a PE transpose
            # (transpose outputs must start at PSUM partition 0; ride the
            # score-psum ring), then free-reduce to the Z scalar.
            zt = pp_s.tile([128, 128], f32, tag="psc", name=f"zt{b}")
            zpT = zt[0:1, 0:128]
            nc.tensor.transpose(zpT, zp, ident)
            zsum = small.tile([1, 1], f32, tag="zsum", name=f"zsum{b}")
            nc.vector.reduce_sum(zsum, zpT, axis=mybir.AxisListType.X)
            rz = small.tile([1, 1], f32, tag="rz", name=f"rz{b}")
            nc.vector.reciprocal(rz, zsum)
            ctx_sb = small.tile([1, H], f32, tag="ctx", name=f"ctx{b}")
            # scale the two halves on different engines so the batch tail
            # (DVE mul + ACT copy-scale) runs in parallel
            nc.vector.tensor_scalar_mul(
                ctx_sb[:, 0:512], st["pcs"][0:1, :], rz)
            nc.scalar.activation(ctx_sb[:, 512:1024], st["pcs"][32:33, :],
                                 AF.Copy, scale=rz)
            eng = nc.sync if b == BPC - 1 else nc.gpsimd
            eng.dma_start(out=ctx_out[b:b + 1, :], in_=ctx_sb)

        pending = []
        for s in range(BPC * NLT):
            b, lt = divmod(s, NLT)
            if lt == 0:
                state[b] = {
                    "expwT": expwT_pool.tile([128, NLCH], bf16, tag="expwT",
                                             name=f"expwT{b}"),
                    "pcs": None,
                    "zacc": small.tile([128, NLT], f32, tag="zacc",
                                       name=f"zacc{b}"),
                }
            st = state[b]

            if b == 0 and lt == 0:
                encTs = encTs_pre
            else:
                encTs = encT_pool.tile([128, KC, 512], f8, tag="encTs")
                nc.sync.dma_start(out=encTs, in_=encT[b, :, lt])
            acc = en_pool.tile([128, 512], f32, tag="acc")
            accb = en_pool.tile([128, 512], bf16, tag="accb")
            for o in range(OC):
                pe = pp_e.tile([128, 512], f32, tag="pe")
                for k2 in range(KC2):
                    nc.tensor.matmul(
                        pe,
                        we_sb[:, 2 * k2:2 * k2 + 2, o * 128:(o + 1) * 128],
                        encTs[:, 2 * k2:2 * k2 + 2, :],
                        start=(k2 == 0), stop=(k2 == KC2 - 1),
                        perf_mode=DR,
                    )
                en = en_pool.tile([128, 512], f32, tag="en")
                nc.scalar.activation(en, pe, AF.Tanh,
                                     bias=hproj_sb[:, o, b:b + 1],
                                     scale=1.0 / W_SCALE)
                # accumulate v-weighted energy on DVE (partition-wise);
                # the last link rounds to bf16 once - it becomes the
                # score-matmul stationary (bf16 gets fast weight load).
                if o == 0:
                    nc.vector.tensor_scalar_mul(acc, en, v_sb[:, 0:1])
                elif o == OC - 1:
                    nc.vector.scalar_tensor_tensor(
                        out=accb, in0=en, scalar=v_sb[:, o:o + 1], in1=acc,
                        op0=mybir.AluOpType.mult, op1=mybir.AluOpType.add)
                else:
                    nc.vector.scalar_tensor_tensor(
                        out=acc, in0=en, scalar=v_sb[:, o:o + 1], in1=acc,
                        op0=mybir.AluOpType.mult, op1=mybir.AluOpType.add)
            # previous slab's ctx matmuls go here: they are dependency-free
            # by now, so they absorb the PE wait for this slab's DVE chain
            # (which the score matmuls below consume as their stationary).
            # The model-time floor (tile_wait_until) keeps the scheduler
            # from splicing them between this slab's e_proj groups - each
            # DR<->bf16 splice costs an exposed ~130ns weight load.
            if len(pending) > 1:
                pb, plt, pencNs = pending.pop(0)
                with tc.tile_wait_until(CTX_W0 + CTX_MS * s):
                    ctx_mms(pb, plt, pencNs)
                    if plt == NLT - 1:
                        finalize(pb)
            # partition-reduce acc with acc as the STATIONARY operand: the
            # scores land transposed ([l%128, lc]) so exp writes the ctx
            # weights directly - no DRAM transpose bounce. Each matmul
            # writes 4 identical columns so the psum dst stays 16B-wide.
            psum_scT = pp_s.tile([128, 128], f32, tag="psc")
            for lc4 in range(4):
                nc.tensor.matmul(psum_scT[:, 4 * lc4:4 * lc4 + 4],
                                 accb[:, lc4 * 128:(lc4 + 1) * 128],
                                 ones4_bf, start=True, stop=True)
            # exp (no max subtraction; scores bounded), Z-partials for free
            nc.scalar.activation(st["expwT"][:, lt * 4:(lt + 1) * 4],
                                 psum_scT[:, 0:16]
                                 .rearrange("p (c f) -> p c f", f=4)[:, :, 0],
                                 AF.Exp,
                                 accum_out=st["zacc"][:, lt:lt + 1])
            # bf16 enc stream for the ctx matmul; alternate DMA rings
            eng = nc.scalar if s % 2 == 0 else nc.gpsimd
            encNs = encN_pool.tile([128, 4, H], bf16, tag="encNs",
                                   name=f"encNs{b}_{lt}")
            eng.dma_start(out=encNs, in_=encN[b, :, lt])
            pending.append((b, lt, encNs))
            if s == BPC * NLT - 3:
                filler_mov = accb
        # keep the PE array HAM-warm through the last slab's exposed
        # DVE-chain drain: dependency-free junk matmuls into an unused
        # partition row of the ctx psum bank. Floors are STAGGERED so the
        # scheduler spreads them across the drain window instead of
        # clumping them into the e_proj stream.
        fill_pcs = state[BPC - 1]["pcs"]
        for g in range(10):
            with tc.tile_wait_until(CTX_W0 + CTX_MS * (BPC * NLT - 0.45
                                                       + 0.08 * g)):
                for f in range(2):
                    nc.tensor.matmul(fill_pcs[96:97, :], ones4_bf[:, 0:1],
                                     filler_mov, start=True, stop=True,
                                     tile_position=(0, 96))
        tail_i = 0
        while pending:
            pb, plt, pencNs = pending.pop(0)
            with tc.tile_wait_until(CTX_W0 + CTX_MS * (BPC * NLT - 2.6 + tail_i)):
                ctx_mms(pb, plt, pencNs)
                if plt == NLT - 1:
                    finalize(pb)
            tail_i += 1

    nc.compile()
    return nc


def _get_nc():
    global _CACHED_NC
    if _CACHED_NC is None:
        _CACHED_NC = _build_kernel()
    return _CACHED_NC


def _make_in_maps(hidden, encoder_outputs, attn_w, attn_b, v_w):
    import ml_dtypes

    f8 = ml_dtypes.float8_e4m3
    bf16 = ml_dtypes.bfloat16

    hidden = np.asarray(hidden, dtype=np.float32)
    encoder_outputs = np.asarray(encoder_outputs, dtype=np.float32)
    attn_w = np.asarray(attn_w, dtype=np.float32)
    attn_b = np.asarray(attn_b, dtype=np.float32)
    v_w = np.asarray(v_w, dtype=np.float32)

    wer = np.ascontiguousarray(
        (attn_w[:, H:] * np.float32(W_SCALE))
        .T.reshape(KC, 128, H).transpose(1, 0, 2)).astype(f8)
    # fold the tiny h_proj = hidden @ w_h^T + b into a per-core bias input
    hproj_pb = hidden @ attn_w[:, :H].T + attn_b     # [B, H]

    enc8 = encoder_outputs.astype(f8)                # cast once, full tensor
    encb = encoder_outputs.astype(bf16)

    in_maps = []
    for c in range(NCORES):
        sl = slice(c * BPC, (c + 1) * BPC)
        # encTr[b, p, lt, k, l] = enc[b, lt*512 + l, k*128 + p]
        encTr = np.ascontiguousarray(
            enc8[sl].reshape(BPC, NLT, 512, KC, 128).transpose(0, 4, 1, 3, 2))
        # encNr[b, p, lt, j, h] = enc[b, lt*512 + j*128 + p, h]
        encNr = np.ascontiguousarray(
            encb[sl].reshape(BPC, NLT, 4, 128, H).transpose(0, 3, 1, 2, 4))
        # smallr: [v chunks | h_proj+b chunks]  (hp[p, o, b] layout)
        hp = hproj_pb[sl].T.reshape(OC, 128, BPC).transpose(1, 0, 2)
        smallr = np.concatenate([
            v_w.reshape(OC, 128).T,
            hp.reshape(128, OC * BPC),
        ], axis=1)
        in_maps.append({
            "encTr": encTr,
            "encNr": encNr,
            "wer": wer,
            "smallr": np.ascontiguousarray(smallr),
        })
    return in_maps


def kernel(hidden, encoder_outputs, attn_w, attn_b, v_w):
    from concourse.bass_utils import run_bass_kernel_spmd

    in_maps = _make_in_maps(hidden, encoder_outputs, attn_w, attn_b, v_w)
    nc = _get_nc()
    res = run_bass_kernel_spmd(nc, in_maps, list(range(NCORES)))
    out = np.concatenate([res.results[c]["ctx"] for c in range(NCORES)], axis=0)
    return out.astype(np.float32)


# revision 38
# speedup vs baseline: 1.1814x; 1.1814x over previous
"""Bahdanau-style attention kernel for Trainium2 (8 NeuronCores, data-parallel).

Computes, for each batch b:
    h_proj = hidden @ w_h^T + attn_b                  # [H]
    e_proj = enc[b] @ w_e^T                           # [L, H]
    energy = tanh(h_proj + e_proj)                    # [L, H]
    scores = energy @ v_w                             # [L]
    weights = softmax(scores)                         # [L]
    context[b] = weights @ enc[b]                     # [H]

Sharding: data-parallel over batch B=32 across 8 cores (4 batches/core).
Params are replicated. The softmax max-subtraction is skipped (scores are
bounded by sum|v| <= 32, exp is safe in fp32); the 1/Z normalization is
folded into the final context scaling.

The dominant e_proj GEMM runs in fp8e4m3 with perf_mode=DoubleRow (2
weights/PE cell, 256-deep contraction per pass -> ~2x matmul throughput).
w_e is host-scaled by 32 to clear the e4m3 subnormal range; the 1/32 is
folded into the tanh activation's scale.

The score partition-reduction uses the v-weighted energy accumulator as
the matmul *stationary* against a ones vector, which lands scores already
transposed ([l%128 on partitions]) so exp() writes the context-matmul
weights directly - no DRAM transpose bounce. The context matmul runs in
bf16 and packs the two h-halves into PE column groups 0/32
(tile_position col-tiling) so the two [1,512] matmuls run concurrently.
Z stays fp32 via the activation accumulator ([128,1] partials reduced at
batch finalization), so the softmax normalization is exact.

Built on bacc.Bacc so compile() runs the TRN2 wait-splitting passes
(move_matmul_waits_to_ldweights / generate_event_semaphores).
"""

import numpy as np

H = 1024
B = 32
L = 2048
NCORES = 8
BPC = B // NCORES          # batches per core = 4
KC = H // 128              # contraction chunks of 128 = 8
KC2 = KC // 2              # DoubleRow chunk pairs = 4
OC = H // 128              # output-feature chunks = 8
NLT = L // 512             # l-tiles of 512 = 4
NLCH = L // 128            # l-chunks of 128 = 16
W_SCALE = 32.0             # host-side w_e scaling (cleared by tanh scale)
CTX_W0 = 0.014             # ms: scheduler-model floor for first ctx block
CTX_MS = 0.0070            # ms: per-slab increment of the ctx floor

_CACHED_NC = None


def _build_kernel():
    from contextlib import ExitStack

    import concourse.tile as tile
    from concourse import bacc
    from concourse import mybir
    from concourse.masks import make_identity

    f32 = mybir.dt.float32
    f32r = mybir.dt.float32r
    f8 = mybir.dt.float8e4
    bf16 = mybir.dt.bfloat16
    AF = mybir.ActivationFunctionType
    DR = mybir.MatmulPerfMode.DoubleRow

    nc = bacc.Bacc("TRN2", target_bir_lowering=False, debug=False,
                   num_devices=NCORES)

    # all inputs host-laid-out so every DMA is contiguous per partition
    encT = nc.dram_tensor("encTr", [BPC, 128, NLT, KC, 512], f8,
                          kind="ExternalInput").ap()
    encN = nc.dram_tensor("encNr", [BPC, 128, NLT, 4, H], bf16,
                          kind="ExternalInput").ap()
    w_eT = nc.dram_tensor("wer", [128, KC, H], f8, kind="ExternalInput").ap()
    smallr = nc.dram_tensor("smallr", [128, OC + OC * BPC], f32,
                            kind="ExternalInput").ap()
    ctx_out = nc.dram_tensor("ctx", [BPC, H], f32, kind="ExternalOutput").ap()

    with tile.TileContext(nc) as tc, ExitStack() as ctx:
        consts = ctx.enter_context(tc.tile_pool(name="consts", bufs=1))
        encT_pool = ctx.enter_context(tc.tile_pool(name="encT", bufs=5))
        encN_pool = ctx.enter_context(tc.tile_pool(name="encN", bufs=3))
        en_pool = ctx.enter_context(tc.tile_pool(name="energy", bufs=6))
        small = ctx.enter_context(tc.tile_pool(name="small", bufs=2))
        expwT_pool = ctx.enter_context(tc.tile_pool(name="expwT", bufs=2))

        # ---- constants ----
        # head loads are spread over all four free DMA rings (sync, vector,
        # scalar, gpsimd) so the first slab + weights land ~4x sooner; each
        # keeps multi-KB per-partition lines for packet efficiency.
        small_sb = consts.tile([128, OC + OC * BPC], f32)
        nc.gpsimd.dma_start(out=small_sb, in_=smallr)
        v_sb = small_sb[:, 0:OC]
        # h_proj + attn_b, host-folded: [128, OC, BPC]
        hproj_sb = small_sb[:, OC:].rearrange("p (o b) -> p o b", b=BPC)
        encTs_pre = encT_pool.tile([128, KC, 512], f8, tag="encTs",
                                   name="encTs_pre")
        nc.sync.dma_start(out=encTs_pre[:, 0:KC // 2, :],
                          in_=encT[0, :, 0, 0:KC // 2, :])
        nc.gpsimd.dma_start(out=encTs_pre[:, KC // 2:, :],
                            in_=encT[0, :, 0, KC // 2:, :])
        we_sb = consts.tile([128, KC, H], f8)            # w_e^T  [h-part, k, o]
        nc.scalar.dma_start(out=we_sb[:, 0:2, :], in_=w_eT[:, 0:2, :])
        nc.scalar.dma_start(out=we_sb[:, 2:KC // 2, :],
                            in_=w_eT[:, 2:KC // 2, :])
        nc.sync.dma_start(out=we_sb[:, KC // 2:KC // 2 + 2, :],
                          in_=w_eT[:, KC // 2:KC // 2 + 2, :])
        nc.gpsimd.dma_start(out=we_sb[:, KC // 2 + 2:, :],
                            in_=w_eT[:, KC // 2 + 2:, :])
        ident = consts.tile([128, 128], f32)
        make_identity(nc, ident)
        ones4_bf = consts.tile([128, 4], bf16)
        nc.vector.memset(ones4_bf, 1.0)
        # warmup weights via a fast vector-queue memset so the HAM warmup
        # isn't gated on make_identity riding the busy gpsimd queue
        warm_w = consts.tile([128, 128], f32)
        nc.vector.memset(warm_w, 0.5)

        with tc.tile_pool(name="pp_pro", bufs=1, space="PSUM") as pp_pro:
            # warm the PE HAM while the weight DMAs stream
            pwarm = pp_pro.tile([128, 128], f32, tag="pwarm")
            for w in range(42):
                nc.tensor.matmul(pwarm, warm_w, warm_w, start=True, stop=True,
                                 skip_group_check=True)

        pp_e = ctx.enter_context(tc.tile_pool(name="pp_e", bufs=6, space="PSUM"))
        pp_s = ctx.enter_context(tc.tile_pool(name="pp_s", bufs=1, space="PSUM"))
        pp_c = ctx.enter_context(tc.tile_pool(name="pp_c", bufs=1, space="PSUM"))

        # ---- main pipeline: flat stream of l-slabs across batches ----
        # ctx matmuls run one slab behind their scores; batch finalization
        # (Z reduce + scale + store) rides behind the next batch's first slab.
        state = {}

        def ctx_mms(b, lt, encNs):
            st = state[b]
            if st["pcs"] is None:
                st["pcs"] = pp_c.tile([128, 512], f32, tag="pc",
                                      name=f"pc{b}")
            for j in range(4):
                lc = lt * 4 + j
                for half in range(2):
                    nc.tensor.matmul(
                        st["pcs"][32 * half:32 * half + 1, :],
                        st["expwT"][:, lc:lc + 1],
                        encNs[:, j, half * 512:(half + 1) * 512],
                        start=(lc == 0), stop=(lc == NLCH - 1),
                        tile_position=(0, 32 * half),
                    )

        def finalize(b):
            st = state.pop(b)
            # Z = sum over [128, NLT] partials: free-reduce on DVE, then a
            # ones-matmul folds the 128 partitions into a [1,1] psum scalar.
            zp = small.tile([128, 1], f32, tag="zp", name=f"zp{b}")
            nc.vector.reduce_sum(zp, st["zacc"], axis=mybir.AxisListType.X)
            # flip the 128 Z-partials onto partition 0 with a PE transpose
            # (transpose outputs must start at PSUM partition 0; ride the
            # score-psum ring), then free-reduce to the Z scalar.
            zt = pp_s.tile([128, 128], f32, tag="psc", name=f"zt{b}")
            zpT = zt[0:1, 0:128]
            nc.tensor.transpose(zpT, zp, ident)
            zsum = small.tile([1, 1], f32, tag="zsum", name=f"zsum{b}")
            nc.vector.reduce_sum(zsum, zpT, axis=mybir.AxisListType.X)
            rz = small.tile([1, 1], f32, tag="rz", name=f"rz{b}")
            nc.vector.reciprocal(rz, zsum)
            ctx_sb = small.tile([1, H], f32, tag="ctx", name=f"ctx{b}")
            # scale the two halves on different engines so the batch tail
            # (DVE mul + ACT copy-scale) runs in parallel
            nc.vector.tensor_scalar_mul(
                ctx_sb[:, 0:512], st["pcs"][0:1, :], rz)
            nc.scalar.activation(ctx_sb[:, 512:1024], st["pcs"][32:33, :],
                                 AF.Copy, scale=rz)
            eng = nc.sync if b == BPC - 1 else nc.gpsimd
            eng.dma_start(out=ctx_out[b:b + 1, :], in_=ctx_sb)

        pending = []
        for s in range(BPC * NLT):
            b, lt = divmod(s, NLT)
            if lt == 0:
                state[b] = {
                    "expwT": expwT_pool.tile([128, NLCH], bf16, tag="expwT",
                                             name=f"expwT{b}"),
                    "pcs": None,
                    "zacc": small.tile([128, NLT], f32, tag="zacc",
                                       name=f"zacc{b}"),
                }
            st = state[b]

            if b == 0 and lt == 0:
                encTs = encTs_pre
            else:
                encTs = encT_pool.tile([128, KC, 512], f8, tag="encTs")
                nc.sync.dma_start(out=encTs, in_=encT[b, :, lt])
            acc = en_pool.tile([128, 512], f32, tag="acc")
            accb = en_pool.tile([128, 512], bf16, tag="accb")
            for o in range(OC):
                pe = pp_e.tile([128, 512], f32, tag="pe")
                for k2 in range(KC2):
                    nc.tensor.matmul(
                        pe,
                        we_sb[:, 2 * k2:2 * k2 + 2, o * 128:(o + 1) * 128],
                        encTs[:, 2 * k2:2 * k2 + 2, :],
                        start=(k2 == 0), stop=(k2 == KC2 - 1),
                        perf_mode=DR,
                    )
                en = en_pool.tile([128, 512], f32, tag="en")
                nc.scalar.activation(en, pe, AF.Tanh,
                                     bias=hproj_sb[:, o, b:b + 1],
                                     scale=1.0 / W_SCALE)
                # accumulate v-weighted energy on DVE (partition-wise);
                # the last link rounds to bf16 once - it becomes the
                # score-matmul stationary (bf16 gets fast weight load).
                if o == 0:
                    nc.vector.tensor_scalar_mul(acc, en, v_sb[:, 0:1])
                elif o == OC - 1:
                    nc.vector.scalar_tensor_tensor(
                        out=accb, in0=en, scalar=v_sb[:, o:o + 1], in1=acc,
                        op0=mybir.AluOpType.mult, op1=mybir.AluOpType.add)
                else:
                    nc.vector.scalar_tensor_tensor(
                        out=acc, in0=en, scalar=v_sb[:, o:o + 1], in1=acc,
                        op0=mybir.AluOpType.mult, op1=mybir.AluOpType.add)
            # previous slab's ctx matmuls go here: they are dependency-free
            # by now, so they absorb the PE wait for this slab's DVE chain
            # (which the score matmuls below consume as their stationary).
            # The model-time floor (tile_wait_until) keeps the scheduler
            # from splicing them between this slab's e_proj groups - each
            # DR<->bf16 splice costs an exposed ~130ns weight load.
            if len(pending) > 1:
                pb, plt, pencNs = pending.pop(0)
                with tc.tile_wait_until(CTX_W0 + CTX_MS * s):
                    ctx_mms(pb, plt, pencNs)
                    if plt == NLT - 1:
                        finalize(pb)
            # partition-reduce acc with acc as the STATIONARY operand: the
            # scores land transposed ([l%128, lc]) so exp writes the ctx
            # weights directly - no DRAM transpose bounce. Each matmul
            # writes 4 identical columns so the psum dst stays 16B-wide.
            psum_scT = pp_s.tile([128, 128], f32, tag="psc")
            for lc4 in range(4):
                nc.tensor.matmul(psum_scT[:, 4 * lc4:4 * lc4 + 4],
                                 accb[:, lc4 * 128:(lc4 + 1) * 128],
                                 ones4_bf, start=True, stop=True)
            # exp (no max subtraction; scores bounded), Z-partials for free
            nc.scalar.activation(st["expwT"][:, lt * 4:(lt + 1) * 4],
                                 psum_scT[:, 0:16]
                                 .rearrange("p (c f) -> p c f", f=4)[:, :, 0],
                                 AF.Exp,
                                 accum_out=st["zacc"][:, lt:lt + 1])
            # bf16 enc stream for the ctx matmul; alternate DMA rings
            eng = nc.scalar if s % 2 == 0 else nc.gpsimd
            encNs = encN_pool.tile([128, 4, H], bf16, tag="encNs",
                                   name=f"encNs{b}_{lt}")
            eng.dma_start(out=encNs, in_=encN[b, :, lt])
            pending.append((b, lt, encNs))
            if s == BPC * NLT - 3:
                filler_mov = accb
        # keep the PE array HAM-warm through the last slab's exposed
        # DVE-chain drain: dependency-free junk matmuls into an unused
        # partition row of the ctx psum bank. Floors are STAGGERED so the
        # scheduler spreads them across the drain window instead of
        # clumping them into the e_proj stream.
        fill_pcs = state[BPC - 1]["pcs"]
        for g in range(10):
            with tc.tile_wait_until(CTX_W0 + CTX_MS * (BPC * NLT - 0.45
                                                       + 0.08 * g)):
                for f in range(2):
                    nc.tensor.matmul(fill_pcs[96:97, :], ones4_bf[:, 0:1],
                                     filler_mov, start=True, stop=True,
                                     tile_position=(0, 96))
        tail_i = 0
        while pending:
            pb, plt, pencNs = pending.pop(0)
            with tc.tile_wait_until(CTX_W0 + CTX_MS * (BPC * NLT - 2.6 + tail_i)):
                ctx_mms(pb, plt, pencNs)
                if plt == NLT - 1:
                    finalize(pb)
            tail_i += 1

    nc.compile()
    return nc


def _get_nc():
    global _CACHED_NC
    if _CACHED_NC is None:
        _CACHED_NC = _build_kernel()
    return _CACHED_NC


def _make_in_maps(hidden, encoder_outputs, attn_w, attn_b, v_w):
    import ml_dtypes

    f8 = ml_dtypes.float8_e4m3
    bf16 = ml_dtypes.bfloat16

    hidden = np.asarray(hidden, dtype=np.float32)
    encoder_outputs = np.asarray(encoder_outputs, dtype=np.float32)
    attn_w = np.asarray(attn_w, dtype=np.float32)
    attn_b = np.asarray(attn_b, dtype=np.float32)
    v_w = np.asarray(v_w, dtype=np.float32)

    wer = np.ascontiguousarray(
        (attn_w[:, H:] * np.float32(W_SCALE))
        .T.reshape(KC, 128, H).transpose(1, 0, 2)).astype(f8)
    # fold the tiny h_proj = hidden @ w_h^T + b into a per-core bias input
    hproj_pb = hidden @ attn_w[:, :H].T + attn_b     # [B, H]

    enc8 = encoder_outputs.astype(f8)                # cast once, full tensor
    encb = encoder_outputs.astype(bf16)

    in_maps = []
    for c in range(NCORES):
        sl = slice(c * BPC, (c + 1) * BPC)
        # encTr[b, p, lt, k, l] = enc[b, lt*512 + l, k*128 + p]
        encTr = np.ascontiguousarray(
            enc8[sl].reshape(BPC, NLT, 512, KC, 128).transpose(0, 4, 1, 3, 2))
        # encNr[b, p, lt, j, h] = enc[b, lt*512 + j*128 + p, h]
        encNr = np.ascontiguousarray(
            encb[sl].reshape(BPC, NLT, 4, 128, H).transpose(0, 3, 1, 2, 4))
        # smallr: [v chunks | h_proj+b chunks]  (hp[p, o, b] layout)
        hp = hproj_pb[sl].T.reshape(OC, 128, BPC).transpose(1, 0, 2)
        smallr = np.concatenate([
            v_w.reshape(OC, 128).T,
            hp.reshape(128, OC * BPC),
        ], axis=1)
        in_maps.append({
            "encTr": encTr,
            "encNr": encNr,
            "wer": wer,
            "smallr": np.ascontiguousarray(smallr),
        })
    return in_maps


def kernel(hidden, encoder_outputs, attn_w, attn_b, v_w):
    from concourse.bass_utils import run_bass_kernel_spmd

    in_maps = _make_in_maps(hidden, encoder_outputs, attn_w, attn_b, v_w)
    nc = _get_nc()
    res = run_bass_kernel_spmd(nc, in_maps, list(range(NCORES)))
    out = np.concatenate([res.results[c]["ctx"] for c in range(NCORES)], axis=0)
    return out.astype(np.float32)
